# revision 43
# baseline (speedup 1.0000x reference)
"""Trainium2 Bass kernel for GQA attention (B=2, T=2048, C=4096, H=32, KV=8, D=128)
with RoPE and causal mask.

Sharding: tensor-parallel over heads across 8 cores. Each core owns 4 Q heads and
their shared KV head: projects q/k/v for those heads, runs causal attention, and
computes a partial output projection; the host sums the 8 partials.

All on-chip layouts are transposed ([feature, token]) so every matmul consumes
natural slices:
  qT/kT/vT = W^T @ x  via lhsT=W-tile [128c, cols], rhs=xT-tile [128c, 512t]
  sT[tk, tq] = kT-tile^T @ qT-chunk   (two 128-key tiles share one [128,1024]
  PSUM pair so a single ACT exp covers both banks)
  pT = exp(sT/sqrt(D) - 10) on ACT; strictly-causal-upper tiles skipped entirely
  softmax denominator: DVE pair+chain adds of the pT tiles, then ONE ones-matmul
  per (head, chunk) broadcasts the partition sum (instead of a ones-matmul per
  key tile, which wasted ~9% of PE time)
  yT[d, tq] += v-tile^T @ pT          (v pre-transposed to [t, d] via PE transpose)
  out[tq, :] += yT_h^T @ wo_h         (accumulate 4 heads in PSUM, evict, DMA out)
A single PSUM pool with dual-role tags spans the whole program so phase
transitions hand off banks tile-by-tile (no pool-boundary barrier, keeps the PE
warm for the HAM clock gate). Output-projection matmul "jobs" are popped from a
queue inside both the attention streams and the projection chunks of the next
batch to keep the in-order PE queue dense.
"""

import os
from collections import deque
from contextlib import ExitStack

import numpy as np
import ml_dtypes

import concourse.bacc as bacc
import concourse.mybir as mybir
import concourse.tile as tile

BF = mybir.dt.bfloat16
F32 = mybir.dt.float32
AFT = mybir.ActivationFunctionType

NCORES = 8
B, T, C = 2, 2048, 4096
H, KV, D = 32, 8, 128
QH = H // NCORES          # 4 q-heads per core
CT = C // 128             # 32 contraction tiles
NCH = T // 512            # 4 query chunks per batch
SKEW = 4                  # q matmuls trail k/v by this many c-tiles
SCALE = 1.0 / float(np.sqrt(D))
EXP_BIAS = -10.0
ROPE_BASE = 10000.0

bf16 = ml_dtypes.bfloat16


def emit_program():
    nc = bacc.Bacc("TRN2", target_bir_lowering=False, debug=False,
                   num_devices=NCORES)

    xT_d = nc.dram_tensor("xT", [C, B * T], BF, kind="ExternalInput").ap()
    # weights pre-arranged on host to [128, ct*cols] so each DMA issues one
    # large contiguous descriptor per partition (256B descriptors starve the
    # DMA rings at startup otherwise)
    wq_d = nc.dram_tensor("wqA", [128, CT * QH * D], BF, kind="ExternalInput").ap()
    wk_d = nc.dram_tensor("wkA", [128, CT * D], BF, kind="ExternalInput").ap()
    wv_d = nc.dram_tensor("wvA", [128, CT * D], BF, kind="ExternalInput").ap()
    wo_d = nc.dram_tensor("woA", [128, QH, C], BF, kind="ExternalInput").ap()
    cos_d = nc.dram_tensor("cosT", [D, T], BF, kind="ExternalInput").ap()
    sin_d = nc.dram_tensor("sinTr", [D, T], BF, kind="ExternalInput").ap()
    alw_d = nc.dram_tensor("allow2", [128, 2, 1024], BF, kind="ExternalInput").ap()
    alwd2_d = nc.dram_tensor("allowd2", [128, 512], BF, kind="ExternalInput").ap()
    id_d = nc.dram_tensor("ident", [128, 128], BF, kind="ExternalInput").ap()
    out_d = nc.dram_tensor("out", [B * T, C], F32, kind="ExternalOutput").ap()

    with tile.TileContext(nc) as tc, ExitStack() as ctx:
        const = ctx.enter_context(tc.tile_pool(name="const", bufs=1))
        act = ctx.enter_context(tc.tile_pool(name="act", bufs=1))
        work = ctx.enter_context(tc.tile_pool(name="work", bufs=1))
        # One PSUM pool for the entire program; 8 banks via dual-role tags:
        #   AB/CD: [128,1024] q-proj pairs  <-> score (dsps) ring
        #   E: k-proj <-> attn@v accumulator (yps)
        #   F: v-proj <-> wo-job ring slot a
        #   G: v-transpose <-> wo-job ring slot b
        #   H: wo-job slot during projections <-> softmax-denominator (dps)
        ps = ctx.enter_context(tc.tile_pool(name="ps", bufs=1, space="PSUM"))

        # ---- weights + tables; first-needed slices go on the fast HWDGE
        # queues so the projection matmuls start ~8us earlier ----
        wq_sb = const.tile([128, CT, QH * D], BF)
        wk_sb = const.tile([128, CT, D], BF)
        wv_sb = const.tile([128, CT, D], BF)
        wqr = wq_d.rearrange("p (ci n) -> p ci n", ci=CT)
        wkr = wk_d.rearrange("p (ci n) -> p ci n", ci=CT)
        wvr = wv_d.rearrange("p (ci n) -> p ci n", ci=CT)
        # each weight tile is fed from a single queue (cross-queue writes to
        # one tile gate the first reader on ALL of them), chunked so the first
        # matmuls wait only on the small leading group
        nc.gpsimd.dma_start(wk_sb[:, 0:4, :], wkr[:, 0:4, :])
        nc.gpsimd.dma_start(wv_sb[:, 0:4, :], wvr[:, 0:4, :])
        nc.gpsimd.dma_start(wk_sb[:, 4:CT, :], wkr[:, 4:CT, :])
        nc.gpsimd.dma_start(wv_sb[:, 4:CT, :], wvr[:, 4:CT, :])
        nc.scalar.dma_start(wq_sb[:, 0:8, :], wqr[:, 0:8, :])
        # small tables next: cos/sin gate the first chunk's rope evictions
        cos_sb = const.tile([D, T], BF)
        nc.scalar.dma_start(cos_sb[:], cos_d)
        sin_sb = const.tile([D, T], BF)
        nc.scalar.dma_start(sin_sb[:], sin_d)
        id_sb = const.tile([128, 128], BF)
        nc.scalar.dma_start(id_sb[:], id_d)
        nc.scalar.dma_start(wq_sb[:, 8:20, :], wqr[:, 8:20, :])
        nc.scalar.dma_start(wq_sb[:, 20:CT, :], wqr[:, 20:CT, :])
        # the masks and wo are only needed from the first attention units on;
        # their DMAs are issued at the end of chunk 0 (see b-loop) to keep the
        # bandwidth-starved first ~50us free for x/wq/wk/wv
        alw_sb = const.tile([128, 2, 1024], BF)
        alwd2_sb = const.tile([128, 512], BF)
        wo_sb = const.tile([128, QH, C], BF)
        onesbf_sb = const.tile([128, 128], BF)
        nc.gpsimd.memset(onesbf_sb[:], 1.0)
        bias_sb = const.tile([128, 1], F32)
        nc.gpsimd.memset(bias_sb[:], EXP_BIAS)

        # per-head / per-chunk tiles: the dependency tracker orders same-tile
        # writes vs reads at whole-tile granularity for these access patterns,
        # so one big tile would serialize every attention read behind the last
        # chunk's eviction write
        qTh = [act.tile([D, T], BF, tag=f"qT{h}", name=f"qT{h}")
               for h in range(QH)]
        kTc = [act.tile([D, 512], BF, tag=f"kT{c}", name=f"kT{c}")
               for c in range(NCH)]
        vTc = [act.tile([D, 512], BF, tag=f"vT{c}", name=f"vT{c}")
               for c in range(NCH)]
        vsbc = [act.tile([128, 4, D], BF, tag=f"v{c}", name=f"vsb{c}")
                for c in range(NCH)]

        wo_jobs = deque()
        pending_tp = deque()
        evict_flip = [0]
        job_flip = [0]

        def pop_job(tag=None, keep=0):
            if len(wo_jobs) > keep:
                if tag is None:
                    tag = ("F", "G")[job_flip[0]]
                    job_flip[0] ^= 1
                wo_jobs.popleft()(tag)

        def make_wo_job(b, j, tl, o, yts):
            def job(tag):
                ops = ps.tile([128, 512], F32, tag=tag, name="ops")
                for h in range(QH):
                    nc.tensor.matmul(
                        ops[:], yts[h][:, 128 * tl:128 * (tl + 1)],
                        wo_sb[:, h, 512 * o:512 * (o + 1)],
                        start=h == 0, stop=h == QH - 1)
                ob = work.tile([128, 512], F32, tag="ob", bufs=4, name="ob")
                # alternate eviction engine to balance DVE/ACT load
                if evict_flip[0] == 0:
                    nc.vector.tensor_copy(ob[:], ops[:])
                else:
                    nc.scalar.copy(ob[:], ops[:])
                evict_flip[0] ^= 1
                r0 = b * T + 512 * j + 128 * tl
                nc.sync.dma_start(out_d[r0:r0 + 128, 512 * o:512 * (o + 1)],
                                  ob[:])
            return job

        def rope_evict(dst, psum, cs):
            # dst = psum * cos + swap_halves(psum) * sin_rot   (bf16 out)
            # swap copies run on ACT so the serial-DVE cost per evict is 3 ops
            sw = work.tile([128, 512], F32, tag="sw", bufs=3, name="sw")
            nc.scalar.copy(sw[0:64, :], psum[64:128, :])
            nc.scalar.copy(sw[64:128, :], psum[0:64, :])
            nc.vector.tensor_mul(sw[:], sw[:], sin_sb[:, cs])
            cst = work.tile([128, 512], F32, tag="cst", bufs=3, name="cst")
            nc.vector.tensor_mul(cst[:], psum[:], cos_sb[:, cs])
            nc.vector.tensor_add(dst, cst[:], sw[:])

        def proj_chunk(b, jc):
            pq01 = ps.tile([128, 1024], F32, tag="AB", name="pq01")
            pq23 = ps.tile([128, 1024], F32, tag="CD", name="pq23")
            pk = ps.tile([128, 512], F32, tag="E", name="pk")
            pv = ps.tile([128, 512], F32, tag="F", name="pv")
            col0 = b * T + 512 * jc
            xts = {}

            def q_mm(h, cq):
                dst = (pq01, pq23)[h // 2]
                half = slice(512 * (h % 2), 512 * (h % 2 + 1))
                nc.tensor.matmul(
                    dst[:, half], wq_sb[:, cq, 128 * h:128 * (h + 1)],
                    xts[cq][:], start=cq == 0, stop=cq == CT - 1)

            for ci in range(CT):
                xt = work.tile([128, 512], BF, tag="xt", bufs=16, name="xt")
                xts[ci] = xt
                nc.sync.dma_start(
                    xt[:], xT_d[128 * ci:128 * (ci + 1), col0:col0 + 512])
                st, sp = ci == 0, ci == CT - 1
                nc.tensor.matmul(pk[:], wk_sb[:, ci, :], xt[:],
                                 start=st, stop=sp)
                nc.tensor.matmul(pv[:], wv_sb[:, ci, :], xt[:],
                                 start=st, stop=sp)
                if jc > 0 and ci in (2, 4, 6, 8):
                    # previous chunk's v transposes, spaced so the ACT evict of
                    # one hides under the k/v matmuls before the next
                    transpose_one(4 * (jc - 1) + ci // 2 - 1, "G")
                if ci >= SKEW:
                    cq = ci - SKEW
                    for h in range(QH):
                        q_mm(h, cq)
                    del xts[cq]
                if ci % 2 == 0:
                    pop_job("H", keep=16)
            cs = slice(512 * jc, 512 * (jc + 1))
            last = jc == NCH - 1
            if not last:
                # pk stopped at ci=CT-1; evicting it now overlaps the q tails
                rope_evict(kTc[jc][:], pk[:], cs)
            # per-head tails: head h's accumulation stops (4-h)*SKEW matmuls
            # before the chunk ends, so its rope eviction overlaps the
            # remaining heads' matmuls and every PSUM slot is free by the time
            # the next phase's first matmul issues
            qhalves = [(pq01, 0), (pq01, 1), (pq23, 0), (pq23, 1)]
            for h in range(QH):
                for cq in range(CT - SKEW, CT):
                    q_mm(h, cq)
                pqt, m = qhalves[h]
                rope_evict(qTh[h][:, cs], pqt[:, 512 * m:512 * (m + 1)], cs)
                if h == 0:
                    if last:
                        rope_evict(kTc[jc][:], pk[:], cs)
                    nc.scalar.copy(vTc[jc][:], pv[:])
            if last:
                for t in range(4):
                    pending_tp.append(4 * jc + t)

        def transpose_one(k, tag):
            tp = ps.tile([128, 128], BF, tag=tag, name="tp")
            nc.tensor.transpose(tp[:], vTc[k // 4][:, 128 * (k % 4):128 * (k % 4 + 1)],
                                id_sb[:])
            nc.scalar.copy(vsbc[k // 4][:, k % 4, :], tp[:])

        def attn_unit(b, j, h, yts, ring=("AB", "CD"), yps_tag="E",
                      pop_tag=None, after_pass1=None):
            K = 4 * j + 4
            KK = K // 2
            # keep a few jobs queued for later, sparser units
            pkeep = 0 if (b == B - 1 and j == NCH - 1) else 6
            if pending_tp:
                transpose_one(pending_tp.popleft(), "G")
            yps = ps.tile([128, 512], F32, tag=yps_tag, name="yps")
            qs = qTh[h][:, 512 * j:512 * (j + 1)]
            qs2 = qTh[h][:, 512 * j + 256:512 * (j + 1)]
            pts = []
            acc = None
            for kk in range(KK):
                if kk == KK - 1:
                    # final key-tile pair sits above the causal diagonal for
                    # the chunk's first 256 queries: compute only q[256:512]
                    sp = ps.tile([128, 512], F32, tag=ring[kk % len(ring)],
                                 name="sp")
                    for m in range(2):
                        k = 2 * kk + m
                        nc.tensor.matmul(
                            sp[:, 256 * m:256 * (m + 1)],
                            kTc[k // 4][:, 128 * (k % 4):128 * (k % 4 + 1)],
                            qs2, start=True, stop=True)
                    pt = work.tile([128, 512], BF, tag="pt5", bufs=3,
                                   name="pt")
                    nc.scalar.activation(pt[:], sp[:], AFT.Exp,
                                         bias=bias_sb[:], scale=SCALE)
                    nc.vector.tensor_mul(pt[:], pt[:], alwd2_sb[:])
                    pr = work.tile([128, 256], BF, tag="pr2", bufs=3,
                                   name="pr")
                    nc.vector.tensor_add(pr[:], pt[:, 0:256], pt[:, 256:512])
                    nc.vector.tensor_add(acc[:, 256:512], acc[:, 256:512],
                                         pr[:])
                    pts.append((pt, True))
                    pop_job(pop_tag, keep=pkeep)
                    continue
                dsps = ps.tile([128, 1024], F32, tag=ring[kk % len(ring)],
                               name="dsps")
                for m in range(2):
                    k = 2 * kk + m
                    nc.tensor.matmul(
                        dsps[:, 512 * m:512 * (m + 1)],
                        kTc[k // 4][:, 128 * (k % 4):128 * (k % 4 + 1)], qs,
                        start=True, stop=True)
                pt = work.tile([128, 1024], BF, tag="pt", bufs=9, name="pt")
                nc.scalar.activation(pt[:], dsps[:], AFT.Exp,
                                     bias=bias_sb[:], scale=SCALE)
                if kk == 2 * j:
                    nc.vector.tensor_mul(pt[:], pt[:], alw_sb[:, 0, :])
                pts.append((pt, False))
                # denominator: pair-sum both halves, chain into acc (bf16)
                if kk == 0:
                    acc = work.tile([128, 512], BF, tag="acc", bufs=2,
                                    name="acc")
                    nc.vector.tensor_add(acc[:], pt[:, 0:512], pt[:, 512:1024])
                else:
                    pr = work.tile([128, 512], BF, tag="pr", bufs=4, name="pr")
                    nc.vector.tensor_add(pr[:], pt[:, 0:512], pt[:, 512:1024])
                    nc.vector.tensor_add(acc[:], acc[:], pr[:])
                pop_job(pop_tag, keep=pkeep)
            if after_pass1 is not None:
                after_pass1()
            for kk in range(KK):
                pt, special = pts[kk]
                for m in range(2):
                    k = 2 * kk + m
                    vs = vsbc[k // 4][:, k % 4, :]
                    if special:
                        nc.tensor.matmul(yps[:, 256:512], vs,
                                         pt[:, 256 * m:256 * (m + 1)],
                                         start=False, stop=(m == 1))
                    else:
                        nc.tensor.matmul(yps[:], vs,
                                         pt[:, 512 * m:512 * (m + 1)],
                                         start=(kk == 0 and m == 0),
                                         stop=False)
                if kk % 2 == 1:
                    pop_job(pop_tag, keep=pkeep)
            dps = ps.tile([128, 512], F32, tag="H", name="dps")
            nc.tensor.matmul(dps[:], onesbf_sb[:], acc[:],
                             start=True, stop=True)
            rec = work.tile([128, 512], F32, tag="rec", bufs=2, name="rec")
            nc.vector.reciprocal_approx_fast(rec[:], dps[:])
            yt = work.tile([128, 512], BF, tag="yt", bufs=8, name="yt")
            nc.vector.tensor_mul(yt[:], yps[:], rec[:])
            yts[h] = yt

        def attn_group(b, j):
            yts = {}
            for h in range(QH):
                attn_unit(b, j, h, yts)
            for tl in range(4):
                for o in range(C // 512):
                    wo_jobs.append(make_wo_job(b, j, tl, o, yts))

        for b in range(B):
            for jc in range(NCH):
                proj_chunk(b, jc)
                if b == 0 and jc == 0:
                    nc.scalar.dma_start(alw_sb[:], alw_d)
                    nc.scalar.dma_start(alwd2_sb[:], alwd2_d)
                    nc.scalar.dma_start(wo_sb[:], wo_d)
                if jc >= 1:
                    # attention group j=jc-1 only needs chunks <= jc-1 plus
                    # chunk jc-1's v transposes (done during chunk jc): emit it
                    # here so its exp-paced stretches are interleaved with the
                    # dense projection stream instead of clumping at the end
                    attn_group(b, jc - 1)
            attn_group(b, NCH - 1)
        while wo_jobs:
            pop_job()

    nc.compile()
    return nc


def host_prep(inputs):
    x = np.asarray(inputs["x"], np.float32)
    mask = np.asarray(inputs["mask"], np.float32)
    wq = np.asarray(inputs["wq"], np.float32)
    wk = np.asarray(inputs["wk"], np.float32)
    wv = np.asarray(inputs["wv"], np.float32)
    wo = np.asarray(inputs["wo"], np.float32)

    xT = np.ascontiguousarray(x.reshape(B * T, C).T).astype(bf16)
    inv = 1.0 / (ROPE_BASE ** (np.arange(0, D, 2, dtype=np.float64) / D))
    freqs = np.arange(T, dtype=np.float64)[:, None] * inv[None, :] * B
    emb = np.concatenate([freqs, freqs], axis=-1)       # [T, D]
    cosT = np.cos(emb).T.astype(np.float32).astype(bf16)
    sinT = np.sin(emb).T.astype(np.float32)
    sinT[: D // 2] *= -1.0
    sinTr = sinT.astype(bf16)
    # allow[p, o, jj] = 1 - mask[jj, 128*o + p]; packed in pairs of key tiles
    # to match the [128, 1024] double-wide exp tiles
    alw = [np.ascontiguousarray((1.0 - mask[0:512, 128 * o:128 * (o + 1)]).T)
           for o in range(4)]
    allow2 = np.stack([np.concatenate([alw[0], alw[1]], axis=1),
                       np.concatenate([alw[2], alw[3]], axis=1)],
                      axis=1).astype(bf16)               # [128, 2, 1024]
    allowd2 = np.concatenate([alw[2][:, 256:512], alw[3][:, 256:512]],
                             axis=1).astype(bf16)        # [128, 512]
    ident = np.eye(128, dtype=np.float32).astype(bf16)

    def flat(w):
        # [C, n] -> [128, CT*n] with partition p holding ctile-major rows
        n = w.shape[1]
        return np.ascontiguousarray(
            w.reshape(CT, 128, n).transpose(1, 0, 2).reshape(128, CT * n))

    common = dict(xT=xT, cosT=cosT, sinTr=sinTr, allow2=allow2,
                  allowd2=allowd2, ident=ident)
    in_maps = []
    for c in range(NCORES):
        m = dict(common)
        m["wqA"] = flat(wq[:, 512 * c:512 * (c + 1)]).astype(bf16)
        m["wkA"] = flat(wk[:, 128 * c:128 * (c + 1)]).astype(bf16)
        m["wvA"] = flat(wv[:, 128 * c:128 * (c + 1)]).astype(bf16)
        m["woA"] = np.ascontiguousarray(
            wo[512 * c:512 * (c + 1), :].reshape(QH, 128, C)
            .transpose(1, 0, 2)).astype(bf16)
        in_maps.append(m)
    return in_maps


def kernel(**inputs) -> np.ndarray:
    from concourse.bass_utils import run_bass_kernel_spmd

    in_maps = host_prep(inputs)
    nc = emit_program()
    trace = bool(os.environ.get("BASS_KERNEL_TRACE"))
    res = run_bass_kernel_spmd(nc, in_maps, core_ids=list(range(NCORES)),
                               trace=trace)
    if trace and res.exec_time_ns is not None:
        print(f"HW exec time: {res.exec_time_ns} ns")
        if res.instructions_and_trace is not None:
            print("trace:", res.instructions_and_trace[1])
    total = np.zeros((B * T, C), np.float32)
    for r in res.results:
        total += r["out"]
    return total.reshape(B, T, C)


# revision 47
# speedup vs baseline: 1.0004x; 1.0004x over previous
"""Trainium2 Bass kernel for GQA attention (B=2, T=2048, C=4096, H=32, KV=8, D=128)
with RoPE and causal mask.

Sharding: tensor-parallel over heads across 8 cores. Each core owns 4 Q heads and
their shared KV head: projects q/k/v for those heads, runs causal attention, and
computes a partial output projection; the host sums the 8 partials.

All on-chip layouts are transposed ([feature, token]) so every matmul consumes
natural slices:
  qT/kT/vT = W^T @ x  via lhsT=W-tile [128c, cols], rhs=xT-tile [128c, 512t]
  sT[tk, tq] = kT-tile^T @ qT-chunk   (two 128-key tiles share one [128,1024]
  PSUM pair so a single ACT exp covers both banks)
  pT = exp(sT/sqrt(D) - 10) on ACT; strictly-causal-upper tiles skipped entirely
  softmax denominator: DVE pair+chain adds of the pT tiles, then ONE ones-matmul
  per (head, chunk) broadcasts the partition sum (instead of a ones-matmul per
  key tile, which wasted ~9% of PE time)
  yT[d, tq] += v-tile^T @ pT          (v pre-transposed to [t, d] via PE transpose)
  out[tq, :] += yT_h^T @ wo_h         (accumulate 4 heads in PSUM, evict, DMA out)
A single PSUM pool with dual-role tags spans the whole program so phase
transitions hand off banks tile-by-tile (no pool-boundary barrier, keeps the PE
warm for the HAM clock gate). Output-projection matmul "jobs" are popped from a
queue inside both the attention streams and the projection chunks of the next
batch to keep the in-order PE queue dense.
"""

import os
from collections import deque
from contextlib import ExitStack

import numpy as np
import ml_dtypes

import concourse.bacc as bacc
import concourse.mybir as mybir
import concourse.tile as tile

BF = mybir.dt.bfloat16
F32 = mybir.dt.float32
AFT = mybir.ActivationFunctionType

NCORES = 8
B, T, C = 2, 2048, 4096
H, KV, D = 32, 8, 128
QH = H // NCORES          # 4 q-heads per core
CT = C // 128             # 32 contraction tiles
NCH = T // 512            # 4 query chunks per batch
SKEW = 4                  # q matmuls trail k/v by this many c-tiles
SCALE = 1.0 / float(np.sqrt(D))
EXP_BIAS = -10.0
ROPE_BASE = 10000.0

bf16 = ml_dtypes.bfloat16


def emit_program():
    nc = bacc.Bacc("TRN2", target_bir_lowering=False, debug=False,
                   num_devices=NCORES)

    xT_d = nc.dram_tensor("xT", [C, B * T], BF, kind="ExternalInput").ap()
    # weights pre-arranged on host to [128, ct*cols] so each DMA issues one
    # large contiguous descriptor per partition (256B descriptors starve the
    # DMA rings at startup otherwise)
    wq_d = nc.dram_tensor("wqA", [128, CT * QH * D], BF, kind="ExternalInput").ap()
    wk_d = nc.dram_tensor("wkA", [128, CT * D], BF, kind="ExternalInput").ap()
    wv_d = nc.dram_tensor("wvA", [128, CT * D], BF, kind="ExternalInput").ap()
    wo_d = nc.dram_tensor("woA", [128, QH, C], BF, kind="ExternalInput").ap()
    cos_d = nc.dram_tensor("cosT", [D, T], BF, kind="ExternalInput").ap()
    sin_d = nc.dram_tensor("sinTr", [D, T], BF, kind="ExternalInput").ap()
    alw_d = nc.dram_tensor("allow2", [128, 2, 1024], BF, kind="ExternalInput").ap()
    alwd2_d = nc.dram_tensor("allowd2", [128, 512], BF, kind="ExternalInput").ap()
    id_d = nc.dram_tensor("ident", [128, 128], BF, kind="ExternalInput").ap()
    out_d = nc.dram_tensor("out", [B * T, C], F32, kind="ExternalOutput").ap()

    with tile.TileContext(nc) as tc, ExitStack() as ctx:
        const = ctx.enter_context(tc.tile_pool(name="const", bufs=1))
        act = ctx.enter_context(tc.tile_pool(name="act", bufs=1))
        work = ctx.enter_context(tc.tile_pool(name="work", bufs=1))
        # One PSUM pool for the entire program; 8 banks via dual-role tags:
        #   AB/CD: [128,1024] q-proj pairs  <-> score (dsps) ring
        #   E: k-proj <-> attn@v accumulator (yps)
        #   F: v-proj <-> wo-job ring slot a
        #   G: v-transpose <-> wo-job ring slot b
        #   H: wo-job slot during projections <-> softmax-denominator (dps)
        ps = ctx.enter_context(tc.tile_pool(name="ps", bufs=1, space="PSUM"))

        # ---- weights + tables; first-needed slices go on the fast HWDGE
        # queues so the projection matmuls start ~8us earlier ----
        wq_sb = const.tile([128, CT, QH * D], BF)
        wk_sb = const.tile([128, CT, D], BF)
        wv_sb = const.tile([128, CT, D], BF)
        wqr = wq_d.rearrange("p (ci n) -> p ci n", ci=CT)
        wkr = wk_d.rearrange("p (ci n) -> p ci n", ci=CT)
        wvr = wv_d.rearrange("p (ci n) -> p ci n", ci=CT)
        # each weight tile is fed from a single queue (cross-queue writes to
        # one tile gate the first reader on ALL of them), chunked so the first
        # matmuls wait only on the small leading group
        nc.gpsimd.dma_start(wk_sb[:, 0:4, :], wkr[:, 0:4, :])
        nc.gpsimd.dma_start(wv_sb[:, 0:4, :], wvr[:, 0:4, :])
        nc.gpsimd.dma_start(wk_sb[:, 4:CT, :], wkr[:, 4:CT, :])
        nc.gpsimd.dma_start(wv_sb[:, 4:CT, :], wvr[:, 4:CT, :])
        nc.scalar.dma_start(wq_sb[:, 0:8, :], wqr[:, 0:8, :])
        # small tables next: cos/sin gate the first chunk's rope evictions
        cos_sb = const.tile([D, T], BF)
        nc.scalar.dma_start(cos_sb[:], cos_d)
        sin_sb = const.tile([D, T], BF)
        nc.scalar.dma_start(sin_sb[:], sin_d)
        id_sb = const.tile([128, 128], BF)
        nc.scalar.dma_start(id_sb[:], id_d)
        nc.scalar.dma_start(wq_sb[:, 8:20, :], wqr[:, 8:20, :])
        nc.scalar.dma_start(wq_sb[:, 20:CT, :], wqr[:, 20:CT, :])
        # the masks and wo are only needed from the first attention units on;
        # their DMAs are issued at the end of chunk 0 (see b-loop) to keep the
        # bandwidth-starved first ~50us free for x/wq/wk/wv
        alw_sb = const.tile([128, 2, 1024], BF)
        alwd2_sb = const.tile([128, 512], BF)
        wo_sb = const.tile([128, QH, C], BF)
        onesbf_sb = const.tile([128, 128], BF)
        nc.gpsimd.memset(onesbf_sb[:], 1.0)
        bias_sb = const.tile([128, 1], F32)
        nc.gpsimd.memset(bias_sb[:], EXP_BIAS)

        # per-head / per-chunk tiles: the dependency tracker orders same-tile
        # writes vs reads at whole-tile granularity for these access patterns,
        # so one big tile would serialize every attention read behind the last
        # chunk's eviction write
        qTh = [act.tile([D, T], BF, tag=f"qT{h}", name=f"qT{h}")
               for h in range(QH)]
        kTc = [act.tile([D, 512], BF, tag=f"kT{c}", name=f"kT{c}")
               for c in range(NCH)]
        vTc = [act.tile([D, 512], BF, tag=f"vT{c}", name=f"vT{c}")
               for c in range(NCH)]
        vsbc = [act.tile([128, 4, D], BF, tag=f"v{c}", name=f"vsb{c}")
                for c in range(NCH)]

        # warmup / filler fodder: zero-dependency matmuls keep the HAM clock
        # gate at full rate through the DMA-bound first chunks
        wu_sb = const.tile([128, 512], BF)
        nc.gpsimd.memset(wu_sb[:], 0.0)

        def dummy_mm():
            du = ps.tile([128, 512], F32, tag="H", name="du")
            nc.tensor.matmul(du[:], onesbf_sb[:], wu_sb[:],
                             start=True, stop=True)

        wo_jobs = deque()
        pending_tp = deque()
        evict_flip = [0]
        job_flip = [0]

        def pop_job(tag=None, keep=0):
            if len(wo_jobs) > keep:
                if tag is None:
                    tag = ("F", "G")[job_flip[0]]
                    job_flip[0] ^= 1
                wo_jobs.popleft()(tag)

        def make_wo_job(b, j, tl, o, yts):
            def job(tag):
                ops = ps.tile([128, 512], F32, tag=tag, name="ops")
                for h in range(QH):
                    nc.tensor.matmul(
                        ops[:], yts[h][:, 128 * tl:128 * (tl + 1)],
                        wo_sb[:, h, 512 * o:512 * (o + 1)],
                        start=h == 0, stop=h == QH - 1)
                ob = work.tile([128, 512], F32, tag="ob", bufs=4, name="ob")
                # alternate eviction engine to balance DVE/ACT load
                if evict_flip[0] == 0:
                    nc.vector.tensor_copy(ob[:], ops[:])
                else:
                    nc.scalar.copy(ob[:], ops[:])
                evict_flip[0] ^= 1
                r0 = b * T + 512 * j + 128 * tl
                nc.sync.dma_start(out_d[r0:r0 + 128, 512 * o:512 * (o + 1)],
                                  ob[:])
            return job

        def rope_evict(dst, psum, cs):
            # dst = psum * cos + swap_halves(psum) * sin_rot   (bf16 out)
            # swap copies run on ACT so the serial-DVE cost per evict is 3 ops
            sw = work.tile([128, 512], F32, tag="sw", bufs=3, name="sw")
            nc.scalar.copy(sw[0:64, :], psum[64:128, :])
            nc.scalar.copy(sw[64:128, :], psum[0:64, :])
            nc.vector.tensor_mul(sw[:], sw[:], sin_sb[:, cs])
            cst = work.tile([128, 512], F32, tag="cst", bufs=3, name="cst")
            nc.vector.tensor_mul(cst[:], psum[:], cos_sb[:, cs])
            nc.vector.tensor_add(dst, cst[:], sw[:])

        def proj_chunk(b, jc):
            pq01 = ps.tile([128, 1024], F32, tag="AB", name="pq01")
            pq23 = ps.tile([128, 1024], F32, tag="CD", name="pq23")
            pk = ps.tile([128, 512], F32, tag="E", name="pk")
            pv = ps.tile([128, 512], F32, tag="F", name="pv")
            col0 = b * T + 512 * jc
            xts = {}

            def q_mm(h, cq):
                dst = (pq01, pq23)[h // 2]
                half = slice(512 * (h % 2), 512 * (h % 2 + 1))
                nc.tensor.matmul(
                    dst[:, half], wq_sb[:, cq, 128 * h:128 * (h + 1)],
                    xts[cq][:], start=cq == 0, stop=cq == CT - 1)

            for ci in range(CT):
                xt = work.tile([128, 512], BF, tag="xt", bufs=16, name="xt")
                xts[ci] = xt
                nc.sync.dma_start(
                    xt[:], xT_d[128 * ci:128 * (ci + 1), col0:col0 + 512])
                st, sp = ci == 0, ci == CT - 1
                nc.tensor.matmul(pk[:], wk_sb[:, ci, :], xt[:],
                                 start=st, stop=sp)
                nc.tensor.matmul(pv[:], wv_sb[:, ci, :], xt[:],
                                 start=st, stop=sp)
                if jc > 0 and ci in (2, 4, 6, 8):
                    # previous chunk's v transposes, spaced so the ACT evict of
                    # one hides under the k/v matmuls before the next
                    transpose_one(4 * (jc - 1) + ci // 2 - 1, "G")
                if b == 0 and (jc == 0 or (jc == 1 and ci < 12)):
                    dummy_mm()
                if ci >= SKEW:
                    cq = ci - SKEW
                    for h in range(QH):
                        q_mm(h, cq)
                    del xts[cq]
                if ci % 2 == 0:
                    pop_job("H", keep=20)
            cs = slice(512 * jc, 512 * (jc + 1))
            last = jc == NCH - 1
            if not last:
                # pk stopped at ci=CT-1; evicting it now overlaps the q tails
                rope_evict(kTc[jc][:], pk[:], cs)
            # per-head tails: head h's accumulation stops (4-h)*SKEW matmuls
            # before the chunk ends, so its rope eviction overlaps the
            # remaining heads' matmuls and every PSUM slot is free by the time
            # the next phase's first matmul issues
            qhalves = [(pq01, 0), (pq01, 1), (pq23, 0), (pq23, 1)]
            for h in range(QH):
                for cq in range(CT - SKEW, CT):
                    q_mm(h, cq)
                pqt, m = qhalves[h]
                rope_evict(qTh[h][:, cs], pqt[:, 512 * m:512 * (m + 1)], cs)
                if h == 0:
                    if last:
                        rope_evict(kTc[jc][:], pk[:], cs)
                    nc.scalar.copy(vTc[jc][:], pv[:])
            if last:
                for t in range(4):
                    pending_tp.append(4 * jc + t)

        def transpose_one(k, tag):
            tp = ps.tile([128, 128], BF, tag=tag, name="tp")
            nc.tensor.transpose(tp[:], vTc[k // 4][:, 128 * (k % 4):128 * (k % 4 + 1)],
                                id_sb[:])
            nc.scalar.copy(vsbc[k // 4][:, k % 4, :], tp[:])

        def attn_unit(b, j, h, yts, ring=("AB", "CD"), yps_tag="E",
                      pop_tag=None, after_pass1=None):
            K = 4 * j + 4
            KK = K // 2
            # keep a few jobs queued for later, sparser units
            pkeep = 0 if (b == B - 1 and j == NCH - 1) else 10
            if pending_tp:
                transpose_one(pending_tp.popleft(), "G")
            yps = ps.tile([128, 512], F32, tag=yps_tag, name="yps")
            qs = qTh[h][:, 512 * j:512 * (j + 1)]
            qs2 = qTh[h][:, 512 * j + 256:512 * (j + 1)]
            pts = []
            acc = None
            for kk in range(KK):
                if kk == KK - 1:
                    # final key-tile pair sits above the causal diagonal for
                    # the chunk's first 256 queries: compute only q[256:512]
                    sp = ps.tile([128, 512], F32, tag=ring[kk % len(ring)],
                                 name="sp")
                    for m in range(2):
                        k = 2 * kk + m
                        nc.tensor.matmul(
                            sp[:, 256 * m:256 * (m + 1)],
                            kTc[k // 4][:, 128 * (k % 4):128 * (k % 4 + 1)],
                            qs2, start=True, stop=True)
                    pt = work.tile([128, 512], BF, tag="pt5", bufs=3,
                                   name="pt")
                    nc.scalar.activation(pt[:], sp[:], AFT.Exp,
                                         bias=bias_sb[:], scale=SCALE)
                    nc.vector.tensor_mul(pt[:], pt[:], alwd2_sb[:])
                    pr = work.tile([128, 256], BF, tag="pr2", bufs=3,
                                   name="pr")
                    nc.vector.tensor_add(pr[:], pt[:, 0:256], pt[:, 256:512])
                    nc.vector.tensor_add(acc[:, 256:512], acc[:, 256:512],
                                         pr[:])
                    pts.append((pt, True))
                    pop_job(pop_tag, keep=pkeep)
                    continue
                dsps = ps.tile([128, 1024], F32, tag=ring[kk % len(ring)],
                               name="dsps")
                for m in range(2):
                    k = 2 * kk + m
                    nc.tensor.matmul(
                        dsps[:, 512 * m:512 * (m + 1)],
                        kTc[k // 4][:, 128 * (k % 4):128 * (k % 4 + 1)], qs,
                        start=True, stop=True)
                pt = work.tile([128, 1024], BF, tag="pt", bufs=9, name="pt")
                nc.scalar.activation(pt[:], dsps[:], AFT.Exp,
                                     bias=bias_sb[:], scale=SCALE)
                if kk == 2 * j:
                    nc.vector.tensor_mul(pt[:], pt[:], alw_sb[:, 0, :])
                pts.append((pt, False))
                # denominator: pair-sum both halves, chain into acc (bf16)
                if kk == 0:
                    acc = work.tile([128, 512], BF, tag="acc", bufs=2,
                                    name="acc")
                    nc.vector.tensor_add(acc[:], pt[:, 0:512], pt[:, 512:1024])
                else:
                    pr = work.tile([128, 512], BF, tag="pr", bufs=4, name="pr")
                    nc.vector.tensor_add(pr[:], pt[:, 0:512], pt[:, 512:1024])
                    nc.vector.tensor_add(acc[:], acc[:], pr[:])
                pop_job(pop_tag, keep=pkeep)
            if after_pass1 is not None:
                after_pass1()
            for kk in range(KK):
                pt, special = pts[kk]
                for m in range(2):
                    k = 2 * kk + m
                    vs = vsbc[k // 4][:, k % 4, :]
                    if special:
                        nc.tensor.matmul(yps[:, 256:512], vs,
                                         pt[:, 256 * m:256 * (m + 1)],
                                         start=False, stop=(m == 1))
                    else:
                        nc.tensor.matmul(yps[:], vs,
                                         pt[:, 512 * m:512 * (m + 1)],
                                         start=(kk == 0 and m == 0),
                                         stop=False)
                if kk % 2 == 1:
                    pop_job(pop_tag, keep=pkeep)
            dps = ps.tile([128, 512], F32, tag="H", name="dps")
            nc.tensor.matmul(dps[:], onesbf_sb[:], acc[:],
                             start=True, stop=True)
            rec = work.tile([128, 512], F32, tag="rec", bufs=2, name="rec")
            nc.vector.reciprocal_approx_fast(rec[:], dps[:])
            yt = work.tile([128, 512], BF, tag="yt", bufs=8, name="yt")
            nc.vector.tensor_mul(yt[:], yps[:], rec[:])
            yts[h] = yt

        def attn_group(b, j):
            yts = {}
            for h in range(QH):
                attn_unit(b, j, h, yts)
            for tl in range(4):
                for o in range(C // 512):
                    wo_jobs.append(make_wo_job(b, j, tl, o, yts))

        # ramp the PE clock gate while the first weight/x DMAs are in flight
        for _ in range(20):
            dummy_mm()

        for b in range(B):
            for jc in range(NCH):
                proj_chunk(b, jc)
                if b == 0 and jc == 0:
                    nc.scalar.dma_start(alw_sb[:], alw_d)
                    nc.scalar.dma_start(alwd2_sb[:], alwd2_d)
                    nc.scalar.dma_start(wo_sb[:], wo_d)
                if jc >= 1:
                    # attention group j=jc-1 only needs chunks <= jc-1 plus
                    # chunk jc-1's v transposes (done during chunk jc): emit it
                    # here so its exp-paced stretches are interleaved with the
                    # dense projection stream instead of clumping at the end
                    attn_group(b, jc - 1)
            attn_group(b, NCH - 1)
        while wo_jobs:
            pop_job()

    nc.compile()
    return nc


def host_prep(inputs):
    x = np.asarray(inputs["x"], np.float32)
    mask = np.asarray(inputs["mask"], np.float32)
    wq = np.asarray(inputs["wq"], np.float32)
    wk = np.asarray(inputs["wk"], np.float32)
    wv = np.asarray(inputs["wv"], np.float32)
    wo = np.asarray(inputs["wo"], np.float32)

    xT = np.ascontiguousarray(x.reshape(B * T, C).T).astype(bf16)
    inv = 1.0 / (ROPE_BASE ** (np.arange(0, D, 2, dtype=np.float64) / D))
    freqs = np.arange(T, dtype=np.float64)[:, None] * inv[None, :] * B
    emb = np.concatenate([freqs, freqs], axis=-1)       # [T, D]
    cosT = np.cos(emb).T.astype(np.float32).astype(bf16)
    sinT = np.sin(emb).T.astype(np.float32)
    sinT[: D // 2] *= -1.0
    sinTr = sinT.astype(bf16)
    # allow[p, o, jj] = 1 - mask[jj, 128*o + p]; packed in pairs of key tiles
    # to match the [128, 1024] double-wide exp tiles
    alw = [np.ascontiguousarray((1.0 - mask[0:512, 128 * o:128 * (o + 1)]).T)
           for o in range(4)]
    allow2 = np.stack([np.concatenate([alw[0], alw[1]], axis=1),
                       np.concatenate([alw[2], alw[3]], axis=1)],
                      axis=1).astype(bf16)               # [128, 2, 1024]
    allowd2 = np.concatenate([alw[2][:, 256:512], alw[3][:, 256:512]],
                             axis=1).astype(bf16)        # [128, 512]
    ident = np.eye(128, dtype=np.float32).astype(bf16)

    def flat(w):
        # [C, n] -> [128, CT*n] with partition p holding ctile-major rows
        n = w.shape[1]
        return np.ascontiguousarray(
            w.reshape(CT, 128, n).transpose(1, 0, 2).reshape(128, CT * n))

    common = dict(xT=xT, cosT=cosT, sinTr=sinTr, allow2=allow2,
                  allowd2=allowd2, ident=ident)
    in_maps = []
    for c in range(NCORES):
        m = dict(common)
        m["wqA"] = flat(wq[:, 512 * c:512 * (c + 1)]).astype(bf16)
        m["wkA"] = flat(wk[:, 128 * c:128 * (c + 1)]).astype(bf16)
        m["wvA"] = flat(wv[:, 128 * c:128 * (c + 1)]).astype(bf16)
        m["woA"] = np.ascontiguousarray(
            wo[512 * c:512 * (c + 1), :].reshape(QH, 128, C)
            .transpose(1, 0, 2)).astype(bf16)
        in_maps.append(m)
    return in_maps


def kernel(**inputs) -> np.ndarray:
    from concourse.bass_utils import run_bass_kernel_spmd

    in_maps = host_prep(inputs)
    nc = emit_program()
    trace = bool(os.environ.get("BASS_KERNEL_TRACE"))
    res = run_bass_kernel_spmd(nc, in_maps, core_ids=list(range(NCORES)),
                               trace=trace)
    if trace and res.exec_time_ns is not None:
        print(f"HW exec time: {res.exec_time_ns} ns")
        if res.instructions_and_trace is not None:
            print("trace:", res.instructions_and_trace[1])
    total = np.zeros((B * T, C), np.float32)
    for r in res.results:
        total += r["out"]
    return total.reshape(B, T, C)


# revision 49
# speedup vs baseline: 1.0185x; 1.0181x over previous
"""Trainium2 Bass kernel for GQA attention (B=2, T=2048, C=4096, H=32, KV=8, D=128)
with RoPE and causal mask.

Sharding: tensor-parallel over heads across 8 cores. Each core owns 4 Q heads and
their shared KV head: projects q/k/v for those heads, runs causal attention, and
computes a partial output projection; the host sums the 8 partials.

All on-chip layouts are transposed ([feature, token]) so every matmul consumes
natural slices:
  qT/kT/vT = W^T @ x  via lhsT=W-tile [128c, cols], rhs=xT-tile [128c, 512t]
  sT[tk, tq] = kT-tile^T @ qT-chunk   (two 128-key tiles share one [128,1024]
  PSUM pair so a single ACT exp covers both banks)
  pT = exp(sT/sqrt(D) - 10) on ACT; strictly-causal-upper tiles skipped entirely
  softmax denominator: DVE pair+chain adds of the pT tiles, then ONE ones-matmul
  per (head, chunk) broadcasts the partition sum (instead of a ones-matmul per
  key tile, which wasted ~9% of PE time)
  yT[d, tq] += v-tile^T @ pT          (v pre-transposed to [t, d] via PE transpose)
  out[tq, :] += yT_h^T @ wo_h         (accumulate 4 heads in PSUM, evict, DMA out)
A single PSUM pool with dual-role tags spans the whole program so phase
transitions hand off banks tile-by-tile (no pool-boundary barrier, keeps the PE
warm for the HAM clock gate). Output-projection matmul "jobs" are popped from a
queue inside both the attention streams and the projection chunks of the next
batch to keep the in-order PE queue dense.
"""

import os
from collections import deque
from contextlib import ExitStack

import numpy as np
import ml_dtypes

import concourse.bacc as bacc
import concourse.mybir as mybir
import concourse.tile as tile

BF = mybir.dt.bfloat16
F32 = mybir.dt.float32
AFT = mybir.ActivationFunctionType

NCORES = 8
B, T, C = 2, 2048, 4096
H, KV, D = 32, 8, 128
QH = H // NCORES          # 4 q-heads per core
CT = C // 128             # 32 contraction tiles
NCH = T // 512            # 4 query chunks per batch
SKEW = 4                  # q matmuls trail k/v by this many c-tiles
SCALE = 1.0 / float(np.sqrt(D))
EXP_BIAS = -10.0
ROPE_BASE = 10000.0

bf16 = ml_dtypes.bfloat16


def emit_program():
    nc = bacc.Bacc("TRN2", target_bir_lowering=False, debug=False,
                   num_devices=NCORES)

    xT_d = nc.dram_tensor("xT", [C, B * T], BF, kind="ExternalInput").ap()
    # weights pre-arranged on host to [128, ct*cols] so each DMA issues one
    # large contiguous descriptor per partition (256B descriptors starve the
    # DMA rings at startup otherwise)
    wq_d = nc.dram_tensor("wqA", [128, CT * QH * D], BF, kind="ExternalInput").ap()
    wk_d = nc.dram_tensor("wkA", [128, CT * D], BF, kind="ExternalInput").ap()
    wv_d = nc.dram_tensor("wvA", [128, CT * D], BF, kind="ExternalInput").ap()
    wo_d = nc.dram_tensor("woA", [128, QH, C], BF, kind="ExternalInput").ap()
    cos_d = nc.dram_tensor("cosT", [D, T], BF, kind="ExternalInput").ap()
    sin_d = nc.dram_tensor("sinTr", [D, T], BF, kind="ExternalInput").ap()
    alw_d = nc.dram_tensor("allow2", [128, 2, 1024], BF, kind="ExternalInput").ap()
    alwd2_d = nc.dram_tensor("allowd2", [128, 512], BF, kind="ExternalInput").ap()
    id_d = nc.dram_tensor("ident", [128, 128], BF, kind="ExternalInput").ap()
    out_d = nc.dram_tensor("out", [B * T, C], F32, kind="ExternalOutput").ap()

    with tile.TileContext(nc) as tc, ExitStack() as ctx:
        const = ctx.enter_context(tc.tile_pool(name="const", bufs=1))
        act = ctx.enter_context(tc.tile_pool(name="act", bufs=1))
        work = ctx.enter_context(tc.tile_pool(name="work", bufs=1))
        # One PSUM pool for the entire program; 8 banks via dual-role tags:
        #   AB/CD: [128,1024] q-proj pairs  <-> score (dsps) ring
        #   E: k-proj <-> attn@v accumulator (yps)
        #   F: v-proj <-> wo-job ring slot a
        #   G: v-transpose <-> wo-job ring slot b
        #   H: wo-job slot during projections <-> softmax-denominator (dps)
        ps = ctx.enter_context(tc.tile_pool(name="ps", bufs=1, space="PSUM"))

        # ---- weights + tables; first-needed slices go on the fast HWDGE
        # queues so the projection matmuls start ~8us earlier ----
        wq_sb = const.tile([128, CT, QH * D], BF)
        wk_sb = const.tile([128, CT, D], BF)
        wv_sb = const.tile([128, CT, D], BF)
        wqr = wq_d.rearrange("p (ci n) -> p ci n", ci=CT)
        wkr = wk_d.rearrange("p (ci n) -> p ci n", ci=CT)
        wvr = wv_d.rearrange("p (ci n) -> p ci n", ci=CT)
        # each weight tile is fed from a single queue (cross-queue writes to
        # one tile gate the first reader on ALL of them), chunked so the first
        # matmuls wait only on the small leading group
        nc.gpsimd.dma_start(wk_sb[:, 0:4, :], wkr[:, 0:4, :])
        nc.gpsimd.dma_start(wv_sb[:, 0:4, :], wvr[:, 0:4, :])
        nc.gpsimd.dma_start(wk_sb[:, 4:CT, :], wkr[:, 4:CT, :])
        nc.gpsimd.dma_start(wv_sb[:, 4:CT, :], wvr[:, 4:CT, :])
        nc.scalar.dma_start(wq_sb[:, 0:8, :], wqr[:, 0:8, :])
        # small tables next: cos/sin gate the first chunk's rope evictions
        cos_sb = const.tile([D, T], BF)
        nc.scalar.dma_start(cos_sb[:], cos_d)
        sin_sb = const.tile([D, T], BF)
        nc.scalar.dma_start(sin_sb[:], sin_d)
        id_sb = const.tile([128, 128], BF)
        nc.scalar.dma_start(id_sb[:], id_d)
        nc.scalar.dma_start(wq_sb[:, 8:20, :], wqr[:, 8:20, :])
        nc.scalar.dma_start(wq_sb[:, 20:CT, :], wqr[:, 20:CT, :])
        # the masks and wo are only needed from the first attention units on;
        # their DMAs are issued at the end of chunk 0 (see b-loop) to keep the
        # bandwidth-starved first ~50us free for x/wq/wk/wv
        alw_sb = const.tile([128, 2, 1024], BF)
        alwd2_sb = const.tile([128, 512], BF)
        wo_sb = const.tile([128, QH, C], BF)
        onesbf_sb = const.tile([128, 128], BF)
        nc.gpsimd.memset(onesbf_sb[:], 1.0)
        bias_sb = const.tile([128, 1], F32)
        nc.gpsimd.memset(bias_sb[:], EXP_BIAS)

        # per-head / per-chunk tiles: the dependency tracker orders same-tile
        # writes vs reads at whole-tile granularity for these access patterns,
        # so one big tile would serialize every attention read behind the last
        # chunk's eviction write
        qTh = [act.tile([D, T], BF, tag=f"qT{h}", name=f"qT{h}")
               for h in range(QH)]
        kTc = [act.tile([D, 512], BF, tag=f"kT{c}", name=f"kT{c}")
               for c in range(NCH)]
        vTc = [act.tile([D, 512], BF, tag=f"vT{c}", name=f"vT{c}")
               for c in range(NCH)]
        vsbc = [act.tile([128, 4, D], BF, tag=f"v{c}", name=f"vsb{c}")
                for c in range(NCH)]

        # warmup / filler fodder: zero-dependency matmuls keep the HAM clock
        # gate at full rate through the DMA-bound first chunks
        wu_sb = const.tile([128, 512], BF)
        nc.gpsimd.memset(wu_sb[:], 0.0)

        def dummy_mm():
            du = ps.tile([128, 512], F32, tag="H", name="du")
            nc.tensor.matmul(du[:], onesbf_sb[:], wu_sb[:],
                             start=True, stop=True)

        wo_jobs = deque()
        pending_tp = deque()
        evict_flip = [0]
        job_flip = [0]

        def pop_job(tag=None, keep=0):
            if len(wo_jobs) > keep:
                if tag is None:
                    tag = ("F", "G")[job_flip[0]]
                    job_flip[0] ^= 1
                wo_jobs.popleft()(tag)

        def make_wo_job(b, j, tl, o, yts):
            def job(tag):
                ops = ps.tile([128, 512], F32, tag=tag, name="ops")
                for h in range(QH):
                    nc.tensor.matmul(
                        ops[:], yts[h][:, 128 * tl:128 * (tl + 1)],
                        wo_sb[:, h, 512 * o:512 * (o + 1)],
                        start=h == 0, stop=h == QH - 1)
                ob = work.tile([128, 512], F32, tag="ob", bufs=4, name="ob")
                # alternate eviction engine to balance DVE/ACT load
                if evict_flip[0] == 0:
                    nc.vector.tensor_copy(ob[:], ops[:])
                else:
                    nc.scalar.copy(ob[:], ops[:])
                evict_flip[0] ^= 1
                r0 = b * T + 512 * j + 128 * tl
                nc.sync.dma_start(out_d[r0:r0 + 128, 512 * o:512 * (o + 1)],
                                  ob[:])
            return job

        def rope_evict(dst, psum, cs):
            # dst = psum * cos + swap_halves(psum) * sin_rot   (bf16 out)
            # swap copies run on ACT so the serial-DVE cost per evict is 3 ops
            sw = work.tile([128, 512], F32, tag="sw", bufs=3, name="sw")
            nc.scalar.copy(sw[0:64, :], psum[64:128, :])
            nc.scalar.copy(sw[64:128, :], psum[0:64, :])
            nc.vector.tensor_mul(sw[:], sw[:], sin_sb[:, cs])
            cst = work.tile([128, 512], F32, tag="cst", bufs=3, name="cst")
            nc.vector.tensor_mul(cst[:], psum[:], cos_sb[:, cs])
            nc.vector.tensor_add(dst, cst[:], sw[:])

        def proj_chunk(b, jc):
            pq01 = ps.tile([128, 1024], F32, tag="AB", name="pq01")
            pq23 = ps.tile([128, 1024], F32, tag="CD", name="pq23")
            pk = ps.tile([128, 512], F32, tag="E", name="pk")
            pv = ps.tile([128, 512], F32, tag="F", name="pv")
            col0 = b * T + 512 * jc
            xts = {}

            def q_mm(h, cq):
                dst = (pq01, pq23)[h // 2]
                half = slice(512 * (h % 2), 512 * (h % 2 + 1))
                nc.tensor.matmul(
                    dst[:, half], wq_sb[:, cq, 128 * h:128 * (h + 1)],
                    xts[cq][:], start=cq == 0, stop=cq == CT - 1)

            for ci in range(CT):
                xt = work.tile([128, 512], BF, tag="xt", bufs=16, name="xt")
                xts[ci] = xt
                nc.sync.dma_start(
                    xt[:], xT_d[128 * ci:128 * (ci + 1), col0:col0 + 512])
                st, sp = ci == 0, ci == CT - 1
                nc.tensor.matmul(pk[:], wk_sb[:, ci, :], xt[:],
                                 start=st, stop=sp)
                nc.tensor.matmul(pv[:], wv_sb[:, ci, :], xt[:],
                                 start=st, stop=sp)
                if jc > 0 and ci in (2, 4, 6, 8):
                    # previous chunk's v transposes, spaced so the ACT evict of
                    # one hides under the k/v matmuls before the next
                    transpose_one(4 * (jc - 1) + ci // 2 - 1, "G")
                if ci >= SKEW:
                    cq = ci - SKEW
                    for h in range(QH):
                        q_mm(h, cq)
                    del xts[cq]
                if ci % 2 == 0 and (b == 1 or jc == NCH - 1):
                    # no pops during b0's early chunks: wo is still streaming
                    # in, and a popped job's matmuls would stall the in-order
                    # PE queue on it
                    pop_job("H", keep=20)
            cs = slice(512 * jc, 512 * (jc + 1))
            last = jc == NCH - 1
            if not last:
                # pk stopped at ci=CT-1; evicting it now overlaps the q tails
                rope_evict(kTc[jc][:], pk[:], cs)
            # per-head tails: head h's accumulation stops (4-h)*SKEW matmuls
            # before the chunk ends, so its rope eviction overlaps the
            # remaining heads' matmuls and every PSUM slot is free by the time
            # the next phase's first matmul issues
            qhalves = [(pq01, 0), (pq01, 1), (pq23, 0), (pq23, 1)]
            for h in range(QH):
                for cq in range(CT - SKEW, CT):
                    q_mm(h, cq)
                pqt, m = qhalves[h]
                rope_evict(qTh[h][:, cs], pqt[:, 512 * m:512 * (m + 1)], cs)
                if h == 0:
                    if last:
                        rope_evict(kTc[jc][:], pk[:], cs)
                    nc.scalar.copy(vTc[jc][:], pv[:])
            if last:
                for t in range(4):
                    pending_tp.append(4 * jc + t)

        def transpose_one(k, tag):
            tp = ps.tile([128, 128], BF, tag=tag, name="tp")
            nc.tensor.transpose(tp[:], vTc[k // 4][:, 128 * (k % 4):128 * (k % 4 + 1)],
                                id_sb[:])
            nc.scalar.copy(vsbc[k // 4][:, k % 4, :], tp[:])

        def attn_unit(b, j, h, yts, ring=("AB", "CD"), yps_tag="E",
                      pop_tag=None, after_pass1=None):
            K = 4 * j + 4
            KK = K // 2
            # keep a few jobs queued for later, sparser units
            pkeep = 0 if (b == B - 1 and j == NCH - 1) else 10
            if pending_tp:
                transpose_one(pending_tp.popleft(), "G")
            yps = ps.tile([128, 512], F32, tag=yps_tag, name="yps")
            qs = qTh[h][:, 512 * j:512 * (j + 1)]
            qs2 = qTh[h][:, 512 * j + 256:512 * (j + 1)]
            pts = []
            acc = None
            for kk in range(KK):
                if kk == KK - 1:
                    # final key-tile pair sits above the causal diagonal for
                    # the chunk's first 256 queries: compute only q[256:512]
                    sp = ps.tile([128, 512], F32, tag=ring[kk % len(ring)],
                                 name="sp")
                    for m in range(2):
                        k = 2 * kk + m
                        nc.tensor.matmul(
                            sp[:, 256 * m:256 * (m + 1)],
                            kTc[k // 4][:, 128 * (k % 4):128 * (k % 4 + 1)],
                            qs2, start=True, stop=True)
                    pt = work.tile([128, 512], BF, tag="pt5", bufs=3,
                                   name="pt")
                    nc.scalar.activation(pt[:], sp[:], AFT.Exp,
                                         bias=bias_sb[:], scale=SCALE)
                    nc.vector.tensor_mul(pt[:], pt[:], alwd2_sb[:])
                    pr = work.tile([128, 256], BF, tag="pr2", bufs=3,
                                   name="pr")
                    nc.vector.tensor_add(pr[:], pt[:, 0:256], pt[:, 256:512])
                    nc.vector.tensor_add(acc[:, 256:512], acc[:, 256:512],
                                         pr[:])
                    pts.append((pt, True))
                    pop_job(pop_tag, keep=pkeep)
                    continue
                dsps = ps.tile([128, 1024], F32, tag=ring[kk % len(ring)],
                               name="dsps")
                for m in range(2):
                    k = 2 * kk + m
                    nc.tensor.matmul(
                        dsps[:, 512 * m:512 * (m + 1)],
                        kTc[k // 4][:, 128 * (k % 4):128 * (k % 4 + 1)], qs,
                        start=True, stop=True)
                pt = work.tile([128, 1024], BF, tag="pt", bufs=9, name="pt")
                nc.scalar.activation(pt[:], dsps[:], AFT.Exp,
                                     bias=bias_sb[:], scale=SCALE)
                if kk == 2 * j:
                    nc.vector.tensor_mul(pt[:], pt[:], alw_sb[:, 0, :])
                pts.append((pt, False))
                # denominator: pair-sum both halves, chain into acc (bf16)
                if kk == 0:
                    acc = work.tile([128, 512], BF, tag="acc", bufs=2,
                                    name="acc")
                    nc.vector.tensor_add(acc[:], pt[:, 0:512], pt[:, 512:1024])
                else:
                    pr = work.tile([128, 512], BF, tag="pr", bufs=4, name="pr")
                    nc.vector.tensor_add(pr[:], pt[:, 0:512], pt[:, 512:1024])
                    nc.vector.tensor_add(acc[:], acc[:], pr[:])
                pop_job(pop_tag, keep=pkeep)
            if after_pass1 is not None:
                after_pass1()
            for kk in range(KK):
                pt, special = pts[kk]
                for m in range(2):
                    k = 2 * kk + m
                    vs = vsbc[k // 4][:, k % 4, :]
                    if special:
                        nc.tensor.matmul(yps[:, 256:512], vs,
                                         pt[:, 256 * m:256 * (m + 1)],
                                         start=False, stop=(m == 1))
                    else:
                        nc.tensor.matmul(yps[:], vs,
                                         pt[:, 512 * m:512 * (m + 1)],
                                         start=(kk == 0 and m == 0),
                                         stop=False)
                if kk % 2 == 1:
                    pop_job(pop_tag, keep=pkeep)
            dps = ps.tile([128, 512], F32, tag="H", name="dps")
            nc.tensor.matmul(dps[:], onesbf_sb[:], acc[:],
                             start=True, stop=True)
            rec = work.tile([128, 512], F32, tag="rec", bufs=2, name="rec")
            nc.vector.reciprocal_approx_fast(rec[:], dps[:])
            yt = work.tile([128, 512], BF, tag="yt", bufs=8, name="yt")
            nc.vector.tensor_mul(yt[:], yps[:], rec[:])
            yts[h] = yt

        def attn_group(b, j):
            yts = {}
            for h in range(QH):
                attn_unit(b, j, h, yts)
            for tl in range(4):
                for o in range(C // 512):
                    wo_jobs.append(make_wo_job(b, j, tl, o, yts))

        # ramp the PE clock gate while the first weight/x DMAs are in flight
        for _ in range(20):
            dummy_mm()

        for b in range(B):
            for jc in range(NCH):
                proj_chunk(b, jc)
                if b == 0 and jc == 0:
                    nc.scalar.dma_start(alw_sb[:], alw_d)
                    nc.scalar.dma_start(alwd2_sb[:], alwd2_d)
                if b == 0 and jc == 1:
                    # 4MB wo load rides behind chunk 2's x stream, clear of the
                    # fully bandwidth-bound first two chunks
                    nc.scalar.dma_start(wo_sb[:], wo_d)
                if jc >= 1:
                    # attention group j=jc-1 only needs chunks <= jc-1 plus
                    # chunk jc-1's v transposes (done during chunk jc): emit it
                    # here so its exp-paced stretches are interleaved with the
                    # dense projection stream instead of clumping at the end
                    attn_group(b, jc - 1)
            attn_group(b, NCH - 1)
        while wo_jobs:
            pop_job()

    nc.compile()
    return nc


def host_prep(inputs):
    x = np.asarray(inputs["x"], np.float32)
    mask = np.asarray(inputs["mask"], np.float32)
    wq = np.asarray(inputs["wq"], np.float32)
    wk = np.asarray(inputs["wk"], np.float32)
    wv = np.asarray(inputs["wv"], np.float32)
    wo = np.asarray(inputs["wo"], np.float32)

    xT = np.ascontiguousarray(x.reshape(B * T, C).T).astype(bf16)
    inv = 1.0 / (ROPE_BASE ** (np.arange(0, D, 2, dtype=np.float64) / D))
    freqs = np.arange(T, dtype=np.float64)[:, None] * inv[None, :] * B
    emb = np.concatenate([freqs, freqs], axis=-1)       # [T, D]
    cosT = np.cos(emb).T.astype(np.float32).astype(bf16)
    sinT = np.sin(emb).T.astype(np.float32)
    sinT[: D // 2] *= -1.0
    sinTr = sinT.astype(bf16)
    # allow[p, o, jj] = 1 - mask[jj, 128*o + p]; packed in pairs of key tiles
    # to match the [128, 1024] double-wide exp tiles
    alw = [np.ascontiguousarray((1.0 - mask[0:512, 128 * o:128 * (o + 1)]).T)
           for o in range(4)]
    allow2 = np.stack([np.concatenate([alw[0], alw[1]], axis=1),
                       np.concatenate([alw[2], alw[3]], axis=1)],
                      axis=1).astype(bf16)               # [128, 2, 1024]
    allowd2 = np.concatenate([alw[2][:, 256:512], alw[3][:, 256:512]],
                             axis=1).astype(bf16)        # [128, 512]
    ident = np.eye(128, dtype=np.float32).astype(bf16)

    def flat(w):
        # [C, n] -> [128, CT*n] with partition p holding ctile-major rows
        n = w.shape[1]
        return np.ascontiguousarray(
            w.reshape(CT, 128, n).transpose(1, 0, 2).reshape(128, CT * n))

    common = dict(xT=xT, cosT=cosT, sinTr=sinTr, allow2=allow2,
                  allowd2=allowd2, ident=ident)
    in_maps = []
    for c in range(NCORES):
        m = dict(common)
        m["wqA"] = flat(wq[:, 512 * c:512 * (c + 1)]).astype(bf16)
        m["wkA"] = flat(wk[:, 128 * c:128 * (c + 1)]).astype(bf16)
        m["wvA"] = flat(wv[:, 128 * c:128 * (c + 1)]).astype(bf16)
        m["woA"] = np.ascontiguousarray(
            wo[512 * c:512 * (c + 1), :].reshape(QH, 128, C)
            .transpose(1, 0, 2)).astype(bf16)
        in_maps.append(m)
    return in_maps


def kernel(**inputs) -> np.ndarray:
    from concourse.bass_utils import run_bass_kernel_spmd

    in_maps = host_prep(inputs)
    nc = emit_program()
    trace = bool(os.environ.get("BASS_KERNEL_TRACE"))
    res = run_bass_kernel_spmd(nc, in_maps, core_ids=list(range(NCORES)),
                               trace=trace)
    if trace and res.exec_time_ns is not None:
        print(f"HW exec time: {res.exec_time_ns} ns")
        if res.instructions_and_trace is not None:
            print("trace:", res.instructions_and_trace[1])
    total = np.zeros((B * T, C), np.float32)
    for r in res.results:
        total += r["out"]
    return total.reshape(B, T, C)
